# revision 24
# baseline (speedup 1.0000x reference)
"""Trainium2 Bass kernel for an 8-expert top-2 MoE layer.

Strategy (expert-parallel, per the sharding hint): the host computes the
tiny gating matmul + softmax + top-2 routing, gathers each expert's
assigned tokens, and ships one expert per NeuronCore. Each core runs the
heavy 2-layer MLP for its expert over its assigned tokens, applies the
gate weights on-device, and the host scatter-adds the two expert
contributions per token.

The MLP matmuls run as fp8(e4m3) DoubleRow pair-matmuls (each
instruction contracts K=256 = 2 k-tiles at half-rate-per-row), with
*residual compensation* to keep accuracy: every operand A is shipped as
a hi/lo pair (A_hi = fp8(A), A_lo = fp8(A - A_hi), same scale), and each
1024-contraction runs three streams

    A_hi @ W_hi  +  A_lo @ W_hi  +  A_hi @ W_lo

which costs 12 pair-matmuls per 128-wide output group (vs 8 full-rate
matmuls for f32r) -> 0.75 cycles/row/layer equivalent, and leaves only
residual-of-residual error (~3e-3 max-rel, gate is 2e-2).

Scales are powers of two folded into host-prepped constants:
  W1 is shipped as fp8(64*W1), so PSUM1 = 64*(x@W1);
  h is evicted as relu(PSUM1 + 64*b1) = 64*h (max ~206 < 240 = e4m3 max)
  via one ACT relu (bias AP), then cast to fp8 (hi) on ACT and the
  residual (lo) computed on DVE;
  W2 is shipped as fp8(128*W2), so PSUM2 = 8192*(h@W2), and the y
  eviction folds b2*8192 and gate/8192 into one (psum + b2') * gate'
  DVE op; y ships bf16.

Schedule: token tiles (<=512, one fp32 PSUM bank) are software-
pipelined as L1(0) L1(1) L2(0) L1(2) L2(1) ... so the h-eviction chain
(ACT relu -> ACT fp8 cast -> DVE residual) of tile t hides under tile
t+1's layer-1 matmuls. x/y use a tile-major DRAM layout (each tile's 8
d-rows contiguous per partition -> >=2KB DMA runs at full model
bandwidth, 128 descriptors per transfer). The first tile is ~296 tokens
so its x lands early but its groups still consume weight strips no
faster than the (HWDGE-serialized) strips arrive. Warm-up matmuls off a
memset tile keep the PE p-state ramping from ~1us with no DMA
dependency; the last tile's output DMA is split in halves so only a
quarter of it trails the final matmul.
"""

import numpy as np

NUM_EXPERTS = 8
TOP_K = 2
D = 1024
S1 = 64.0     # W1/h scale
S2 = 128.0    # W2 scale (gate folds 1/(S1*S2))

_prog_cache = {}


def _plan_tiles(max_load):
    """Token-tile sizes covering max_load: a ~296-token first tile (early
    x arrival without starving on weight strips), then 512s, then a tail
    rounded to a multiple of 8 (fp32r gate matmul ISA restriction)."""
    r8 = lambda v: -(-v // 8) * 8
    tiles = []
    rest = max_load
    for first in (296, 384):
        if rest <= 0:
            break
        take = min(first, rest)
        tiles.append(r8(take))
        rest -= take
    n512, rem = divmod(max(rest, 0), 512)
    tiles += [512] * n512
    if rem:
        tiles.append(r8(rem))
    return sum(tiles), tiles


def _build_program(tile_plan):
    """Build the per-core Bass program: one expert's MLP over C tokens."""
    from contextlib import ExitStack

    import concourse.tile as tile
    from concourse import bacc, mybir

    f32 = mybir.dt.float32
    f32r = mybir.dt.float32r
    f8 = mybir.dt.float8e4
    bf16 = mybir.dt.bfloat16
    DR = mybir.MatmulPerfMode.DoubleRow
    ADD = mybir.AluOpType.add
    MULT = mybir.AluOpType.mult
    RELU = mybir.ActivationFunctionType.Relu
    COPY = mybir.ActivationFunctionType.Copy

    C, tok_tiles = tile_plan

    nc = bacc.Bacc("TRN2", target_bir_lowering=False, debug=False,
                   num_devices=NUM_EXPERTS)

    # host-packed layouts (see _make_in_maps), all e4m3 except consts:
    #   xh/xl: [128, 8*C] tile-major: cols [8*pos_t + d*TT_t + c]
    #          = q(x_gathered[pos_t + c, d*128 + p]) hi/lo
    #   w1:  [8, 128, 2, 8, 128]  w1[j, p, v, d, r] = q(64*W1[d*128+p, j*128+r])
    #   w2:  [8, 128, 2, 8, 128]  w2[o, p, v, j, r] = q(128*W2[j*128+p, o*128+r])
    #   bb:  [128, 16] f32        [64*b1 | 8192*b2] per-partition
    #   go:  [1, C+128] f32r      [gate row / 8192 | ones row]
    #   yT:  [128, 8*C] bf16      tile-major like xh/xl, gated y
    xh_d = nc.dram_tensor("xh", [128, 8 * C], f8, kind="ExternalInput").ap()
    xl_d = nc.dram_tensor("xl", [128, 8 * C], f8, kind="ExternalInput").ap()
    w1_d = nc.dram_tensor("w1", [8, 128, 2, 8, 128], f8, kind="ExternalInput").ap()
    w2_d = nc.dram_tensor("w2", [8, 128, 2, 8, 128], f8, kind="ExternalInput").ap()
    bb_d = nc.dram_tensor("bb", [128, 16], f32, kind="ExternalInput").ap()
    gb_d = nc.dram_tensor("gb", [128, C], f32, kind="ExternalInput").ap()
    yT_d = nc.dram_tensor("yT", [128, 8 * C], bf16, kind="ExternalOutput").ap()

    with tile.TileContext(nc) as tc, ExitStack() as ctx:
        wpool = ctx.enter_context(tc.tile_pool(name="w", bufs=1))
        cpool = ctx.enter_context(tc.tile_pool(name="const", bufs=1))
        xpool = ctx.enter_context(tc.tile_pool(name="x", bufs=2))
        hxpool = ctx.enter_context(tc.tile_pool(name="hx", bufs=3))
        hpool = ctx.enter_context(tc.tile_pool(name="h", bufs=2))
        ypool = ctx.enter_context(tc.tile_pool(name="y", bufs=2))
        gpool = ctx.enter_context(tc.tile_pool(name="g", bufs=2))
        php = ctx.enter_context(tc.tile_pool(name="ph", bufs=3, space="PSUM"))
        pyp = ctx.enter_context(tc.tile_pool(name="py", bufs=4, space="PSUM"))
        pwp = ctx.enter_context(tc.tile_pool(name="pw", bufs=1, space="PSUM"))

        # PE warm-up fed by a small memset (no DMA dependency): dummy bf16
        # matmuls keep the PE busy from ~1us so the cost-model p-state
        # reaches full speed right as the first real matmuls arrive. A
        # dummy relu warms the ACT function table (1.3us load) in the
        # shadow of the DMA ramp.
        wsrc = cpool.tile([1, 256], bf16, tag="wsrc")
        nc.vector.memset(wsrc[:], 1.0)
        dummy = cpool.tile([1, 128], bf16, tag="dummy")
        nc.scalar.activation(dummy[:], wsrc[0:1, 0:128],
                             mybir.ActivationFunctionType.Relu,
                             bias=wsrc[0:1, 0:1], scale=1.0)
        warm = pwp.tile([128, 256], f32, tag="warm")
        for _ in range(18):
            nc.tensor.matmul(warm[:], wsrc[:, 0:128], wsrc[:, 0:256],
                             start=True, stop=True)

        # DMA emission in consumption order (transfers serialize on the
        # DMA bus and dispatches on HWDGE at ~650ns each): w1 strip 0 and
        # tile-0 x first, consts, remaining w1 strips, tile-1 x, w2 strips
        TT0 = tok_tiles[0]
        w1_sb = [None] * 8
        w1_first = wpool.tile([128, 2, 8, 128], f8, tag="w1_0")
        nc.sync.dma_start(w1_first[:], w1_d[0])
        w1_sb[0] = w1_first
        xh0 = xpool.tile([128, 8, TT0], f8, tag="xh")
        nc.sync.dma_start(xh0[:], xh_d[:, 0:8 * TT0])
        xl0 = xpool.tile([128, 8, TT0], f8, tag="xl")
        nc.sync.dma_start(xl0[:], xl_d[:, 0:8 * TT0])
        bb_sb = cpool.tile([128, 16], f32, tag="bb")
        nc.sync.dma_start(bb_sb[:], bb_d[:])
        for j in range(1, 8):
            w1_strip = wpool.tile([128, 2, 8, 128], f8, tag=f"w1_{j}")
            nc.sync.dma_start(w1_strip[:], w1_d[j])
            w1_sb[j] = w1_strip

        g_tiles = [None] * len(tok_tiles)
        x_tiles = [None] * len(tok_tiles)
        x_tiles[0] = (xh0, xl0)
        if len(tok_tiles) > 1:
            TT1 = tok_tiles[1]
            sl1 = slice(8 * TT0, 8 * (TT0 + TT1))
            xh1 = xpool.tile([128, 8, TT1], f8, tag="xh")
            nc.sync.dma_start(xh1[:], xh_d[:, sl1])
            xl1 = xpool.tile([128, 8, TT1], f8, tag="xl")
            nc.sync.dma_start(xl1[:], xl_d[:, sl1])
            x_tiles[1] = (xh1, xl1)
        gb0 = gpool.tile([128, TT0], f32, tag="gbc")
        nc.sync.dma_start(gb0[:], gb_d[:, 0:TT0])
        g_tiles[0] = gb0
        if len(tok_tiles) > 1:
            gb1 = gpool.tile([128, tok_tiles[1]], f32, tag="gbc")
            nc.sync.dma_start(gb1[:], gb_d[:, TT0:TT0 + tok_tiles[1]])
            g_tiles[1] = gb1
        w2_sb = [None] * 8
        for o in range(8):
            w2_strip = wpool.tile([128, 2, 8, 128], f8, tag=f"w2_{o}")
            nc.sync.dma_start(w2_strip[:], w2_d[o])
            w2_sb[o] = w2_strip

        tile_pos = np.cumsum([0] + tok_tiles).tolist()
        ntile = len(tok_tiles)
        h_tiles = [None] * ntile
        assert len(g_tiles) == ntile

        def emit_l1(t):
            """Layer 1 + gate broadcast of tile t; leaves h8/hl8 + g_bc."""
            TT = tok_tiles[t]

            # prefetch x for tile t+1 (tiles 0 and 1 issued upfront)
            nt = t + 1
            if nt < ntile and x_tiles[nt] is None:
                TTn = tok_tiles[nt]
                nsl = slice(8 * tile_pos[nt], 8 * (tile_pos[nt] + TTn))
                xh_p = xpool.tile([128, 8, TTn], f8, tag="xh")
                nc.sync.dma_start(xh_p[:], xh_d[:, nsl])
                xl_p = xpool.tile([128, 8, TTn], f8, tag="xl")
                nc.sync.dma_start(xl_p[:], xl_d[:, nsl])
                x_tiles[nt] = (xh_p, xl_p)
                gb_p = gpool.tile([128, TTn], f32, tag="gbc")
                nc.sync.dma_start(gb_p[:],
                                  gb_d[:, tile_pos[nt]:tile_pos[nt] + TTn])
                g_tiles[nt] = gb_p

            xh_sb, xl_sb = x_tiles[t]

            # layer 1: 64*h^T[j] = relu(64*sum_d W1[d,j]^T x^T[d] + 64*b1[j])
            # 3 fp8 DoubleRow streams: xh@W1h + xl@W1h + xh@W1l
            h8 = [hpool.tile([128, 2, TT], f8, tag=f"h8_{q}", name=f"h8_{q}")
                  for q in range(4)]
            hl8 = [hpool.tile([128, 2, TT], f8, tag=f"hl8_{q}", name=f"hl8_{q}")
                   for q in range(4)]
            for j in range(8):
                # full-bank PSUM tile: a start=True matmul clears the whole
                # 2KB zero region, so sub-bank tiles must not share banks
                ph_t = php.tile([128, 512], f32, tag="ph")
                ph = ph_t[:, 0:TT]
                n = 0
                for v, xs in ((0, xh_sb), (1, xh_sb), (0, xl_sb)):
                    for q in range(4):
                        nc.tensor.matmul(ph,
                                         w1_sb[j][:, v, 2 * q:2 * q + 2, :],
                                         xs[:, 2 * q:2 * q + 2, :],
                                         start=(n == 0), stop=(n == 11),
                                         perf_mode=DR)
                        n += 1
                hx32 = hxpool.tile([128, TT], f32, tag="hx32")
                nc.scalar.activation(hx32[:], ph, RELU,
                                     bias=bb_sb[:, j:j + 1], scale=1.0)
                h8s = h8[j // 2][:, j % 2, :]
                nc.scalar.activation(h8s, hx32[:], COPY)
                nc.vector.scalar_tensor_tensor(hl8[j // 2][:, j % 2, :],
                                               h8s, -1.0, hx32[:],
                                               op0=MULT, op1=ADD)
            h_tiles[t] = (h8, hl8)

        def emit_l2(t):
            """Gate broadcast + layer 2 + output DMA of tile t."""
            TT = tok_tiles[t]
            h8, hl8 = h_tiles[t]
            last = t == ntile - 1

            g_bc = g_tiles[t]

            # layer 2 + gate: y^T[o] = (sum_j W2[j,o]^T h^T[j] + b2[o]) * g
            # 3 fp8 DoubleRow streams: h8@W2h + hl8@W2h + h8@W2l
            ybig = ypool.tile([128, 8, TT], bf16, tag="y")
            base = 8 * tile_pos[t]
            # the last tile's o=7 group runs as two half-token groups so
            # only a half-group eviction + DMA trails the final matmul
            Ta = (TT // 2 + 7) // 8 * 8
            subs = [(o, slice(0, TT)) for o in range(8)]
            if last:
                subs[7] = (7, slice(0, Ta))
                subs.append((7, slice(Ta, TT)))
            for o, sl in subs:
                TTs = sl.stop - sl.start
                py_t = pyp.tile([128, 512], f32, tag="py")
                py = py_t[:, 0:TTs]
                n = 0
                for v, hs in ((0, h8), (0, hl8), (1, h8)):
                    for q in range(4):
                        nc.tensor.matmul(py,
                                         w2_sb[o][:, v, 2 * q:2 * q + 2, :],
                                         hs[q][:, :, sl],
                                         start=(n == 0), stop=(n == 11),
                                         perf_mode=DR)
                        n += 1
                nc.vector.scalar_tensor_tensor(ybig[:, o, sl], py,
                                               bb_sb[:, 8 + o:9 + o],
                                               g_bc[:, sl], op0=ADD, op1=MULT)
                # dispatch output pieces as their o-groups complete so the
                # transfers drain off the bus before the program tail
                if o == 3:
                    nc.sync.dma_start(yT_d[:, base:base + 4 * TT],
                                      ybig[:, 0:4, :])
                if last and o == 6:
                    nc.sync.dma_start(yT_d[:, base + 4 * TT:base + 7 * TT],
                                      ybig[:, 4:7, :])
                if last and o == 7 and sl.stop == Ta:
                    nc.sync.dma_start(
                        yT_d[:, base + 7 * TT:base + 7 * TT + Ta],
                        ybig[:, 7, 0:Ta])
            if last:
                nc.sync.dma_start(yT_d[:, base + 7 * TT + Ta:base + 8 * TT],
                                  ybig[:, 7, Ta:TT])
            else:
                nc.sync.dma_start(yT_d[:, base + 4 * TT:base + 8 * TT],
                                  ybig[:, 4:8, :])

        # software pipeline: layer 1 of tile t+1 runs (on PE) before layer
        # 2 of tile t, so the h-eviction chain (ACT relu -> ACT fp8 cast ->
        # DVE residual) of tile t hides under tile t+1's layer-1 matmuls.
        emit_l1(0)
        for t in range(1, ntile):
            emit_l1(t)
            emit_l2(t - 1)
        emit_l2(ntile - 1)

    nc.compile()
    return nc


def _route(x, Wg, bg):
    """Host gating: fp32 softmax + top-2, matching jax.lax.top_k semantics."""
    logits = x @ Wg + bg
    m = logits.max(axis=1, keepdims=True)
    e = np.exp(logits - m)
    gates = e / e.sum(axis=1, keepdims=True)
    # stable argsort on negated values = ties broken by lower index (jax)
    order = np.argsort(-gates, axis=1, kind="stable")[:, :TOP_K]
    return gates, order


def _q8(a):
    import ml_dtypes
    return np.asarray(a).astype(ml_dtypes.float8_e4m3)


def _pack_w(W, scale):
    """[1024,1024] -> [8, 128, 2, 8, 128] hi/lo fp8 strips.

    out[s, p, v, d, r] = q_v(scale * W[d*128+p, s*128+r])
    """
    Ws = (W * scale).astype(np.float32)
    Wh = _q8(Ws)
    Wl = _q8(Ws - Wh.astype(np.float32))
    packs = []
    for Wv in (Wh, Wl):
        # [d, p, s, r] -> [s, p, d, r]
        packs.append(Wv.reshape(8, 128, 8, 128).transpose(2, 1, 0, 3))
    # -> [s, p, v, d, r]
    return np.ascontiguousarray(np.stack(packs, axis=2))


def _pack_x_tiles(xq, toks, tok_tiles, C):
    """Gather + transpose + tile-major pack: [128, 8*C] fp8."""
    out = np.zeros((128, 8 * C), dtype=xq.dtype)
    ne = len(toks)
    pos = 0
    for TT in tok_tiles:
        take = toks[pos:pos + TT]
        if len(take):
            # [nt, 1024] -> [128, 8, nt]
            seg = xq[take].T.reshape(8, 128, len(take)).transpose(1, 0, 2)
            blk = out[:, 8 * pos:8 * (pos + TT)].reshape(128, 8, TT)
            blk[:, :, :len(take)] = seg
        pos += TT
    return out


def _make_in_maps(x, W1, b1, W2, b2, gates, order, tok_lists, C, tok_tiles):
    xh_full = _q8(x)
    xl_full = _q8(x - xh_full.astype(np.float32))
    in_maps = []
    for e in range(NUM_EXPERTS):
        toks = tok_lists[e]
        g_e = np.zeros(C, dtype=np.float32)
        g_e[:len(toks)] = gates[toks, e] / (S1 * S2)
        in_maps.append({
            "xh": _pack_x_tiles(xh_full, toks, tok_tiles, C),
            "xl": _pack_x_tiles(xl_full, toks, tok_tiles, C),
            "w1": _pack_w(W1[e], S1),
            "w2": _pack_w(W2[e], S2),
            "bb": np.ascontiguousarray(np.concatenate(
                [(S1 * b1[e]).reshape(8, 128).T,
                 (S1 * S2 * b2[e]).reshape(8, 128).T], axis=1)),
            "gb": np.ascontiguousarray(
                np.broadcast_to(g_e, (128, C))),
        })
    return in_maps


def kernel(x, W1, b1, W2, b2, Wg, bg):
    from concourse import bass_utils

    x = np.ascontiguousarray(np.asarray(x, dtype=np.float32))
    W1 = np.asarray(W1, dtype=np.float32)
    b1 = np.asarray(b1, dtype=np.float32)
    W2 = np.asarray(W2, dtype=np.float32)
    b2 = np.asarray(b2, dtype=np.float32)
    Wg = np.asarray(Wg, dtype=np.float32)
    bg = np.asarray(bg, dtype=np.float32)
    n = x.shape[0]

    gates, order = _route(x, Wg, bg)
    tok_lists = [np.where((order == e).any(axis=1))[0] for e in range(NUM_EXPERTS)]
    max_load = max(len(t) for t in tok_lists)
    C, tok_tiles = _plan_tiles(max_load)

    key = (C, tuple(tok_tiles))
    if key not in _prog_cache:
        _prog_cache[key] = _build_program((C, tok_tiles))
    nc = _prog_cache[key]

    in_maps = _make_in_maps(x, W1, b1, W2, b2, gates, order, tok_lists, C,
                            tok_tiles)
    res = bass_utils.run_bass_kernel_spmd(nc, in_maps, list(range(NUM_EXPERTS)))
    # yT result: tile-major [128, 8*C] bf16 -> [E, 128, 8, C] f32
    yT_all = np.empty((NUM_EXPERTS, 128, 8, C), dtype=np.float32)
    for e in range(NUM_EXPERTS):
        flat = res.results[e]["yT"].astype(np.float32)
        pos = 0
        for TT in tok_tiles:
            yT_all[e, :, :, pos:pos + TT] = (
                flat[:, 8 * pos:8 * (pos + TT)].reshape(128, 8, TT))
            pos += TT

    # scatter-add the two expert contributions per token (already gated)
    slot = np.zeros((NUM_EXPERTS, n), dtype=np.int64)
    for e in range(NUM_EXPERTS):
        slot[e, tok_lists[e]] = np.arange(len(tok_lists[e]))
    rows = np.arange(n)
    out = np.zeros((n, D), dtype=np.float32)
    for k in range(TOP_K):
        ek = order[:, k]
        picked = yT_all[ek, :, :, slot[ek, rows]]   # [n, 128, 8]
        out += picked.transpose(0, 2, 1).reshape(n, D)
    return out


# revision 25
# speedup vs baseline: 1.0037x; 1.0037x over previous
"""Trainium2 Bass kernel for an 8-expert top-2 MoE layer.

Strategy (expert-parallel, per the sharding hint): the host computes the
tiny gating matmul + softmax + top-2 routing, gathers each expert's
assigned tokens, and ships one expert per NeuronCore. Each core runs the
heavy 2-layer MLP for its expert over its assigned tokens, applies the
gate weights on-device, and the host scatter-adds the two expert
contributions per token.

The MLP matmuls run as fp8(e4m3) DoubleRow pair-matmuls (each
instruction contracts K=256 = 2 k-tiles at half-rate-per-row), with
*residual compensation* to keep accuracy: every operand A is shipped as
a hi/lo pair (A_hi = fp8(A), A_lo = fp8(A - A_hi), same scale), and each
1024-contraction runs three streams

    A_hi @ W_hi  +  A_lo @ W_hi  +  A_hi @ W_lo

which costs 12 pair-matmuls per 128-wide output group (vs 8 full-rate
matmuls for f32r) -> 0.75 cycles/row/layer equivalent, and leaves only
residual-of-residual error (~3e-3 max-rel, gate is 2e-2).

Scales are powers of two folded into host-prepped constants:
  W1 is shipped as fp8(64*W1), so PSUM1 = 64*(x@W1);
  h is evicted as relu(PSUM1 + 64*b1) = 64*h (max ~206 < 240 = e4m3 max)
  via one ACT relu (bias AP), then cast to fp8 (hi) on ACT and the
  residual (lo) computed on DVE;
  W2 is shipped as fp8(128*W2), so PSUM2 = 8192*(h@W2), and the y
  eviction folds b2*8192 and gate/8192 into one (psum + b2') * gate'
  DVE op; y ships bf16.

Schedule: token tiles (<=512, one fp32 PSUM bank) are software-
pipelined as L1(0) L1(1) L2(0) L1(2) L2(1) ... so the h-eviction chain
(ACT relu -> ACT fp8 cast -> DVE residual) of tile t hides under tile
t+1's layer-1 matmuls. x/y use a tile-major DRAM layout (each tile's 8
d-rows contiguous per partition -> >=2KB DMA runs at full model
bandwidth, 128 descriptors per transfer). The first tile is ~296 tokens
so its x lands early but its groups still consume weight strips no
faster than the (HWDGE-serialized) strips arrive. Warm-up matmuls off a
memset tile keep the PE p-state ramping from ~1us with no DMA
dependency; the last tile's output DMA is split in halves so only a
quarter of it trails the final matmul.
"""

import numpy as np

NUM_EXPERTS = 8
TOP_K = 2
D = 1024
S1 = 64.0     # W1/h scale
S2 = 128.0    # W2 scale (gate folds 1/(S1*S2))

_prog_cache = {}


def _plan_tiles(max_load):
    """Token-tile sizes covering max_load: a ~296-token first tile (early
    x arrival without starving on weight strips), then 512s, then a tail
    rounded to a multiple of 8 (fp32r gate matmul ISA restriction)."""
    r8 = lambda v: -(-v // 8) * 8
    tiles = []
    rest = max_load
    for first in (296, 384):
        if rest <= 0:
            break
        take = min(first, rest)
        tiles.append(r8(take))
        rest -= take
    n512, rem = divmod(max(rest, 0), 512)
    tiles += [512] * n512
    if rem:
        tiles.append(r8(rem))
    return sum(tiles), tiles


def _build_program(tile_plan):
    """Build the per-core Bass program: one expert's MLP over C tokens."""
    from contextlib import ExitStack

    import concourse.tile as tile
    from concourse import bacc, mybir

    f32 = mybir.dt.float32
    f32r = mybir.dt.float32r
    f8 = mybir.dt.float8e4
    bf16 = mybir.dt.bfloat16
    DR = mybir.MatmulPerfMode.DoubleRow
    ADD = mybir.AluOpType.add
    MULT = mybir.AluOpType.mult
    RELU = mybir.ActivationFunctionType.Relu
    COPY = mybir.ActivationFunctionType.Copy

    C, tok_tiles = tile_plan

    nc = bacc.Bacc("TRN2", target_bir_lowering=False, debug=False,
                   num_devices=NUM_EXPERTS)

    # host-packed layouts (see _make_in_maps), all e4m3 except consts:
    #   xh/xl: [128, 8*C] tile-major: cols [8*pos_t + d*TT_t + c]
    #          = q(x_gathered[pos_t + c, d*128 + p]) hi/lo
    #   w1:  [8, 128, 2, 8, 128]  w1[j, p, v, d, r] = q(64*W1[d*128+p, j*128+r])
    #   w2:  [8, 128, 2, 8, 128]  w2[o, p, v, j, r] = q(128*W2[j*128+p, o*128+r])
    #   bb:  [128, 16] f32        [64*b1 | 8192*b2] per-partition
    #   go:  [1, C+128] f32r      [gate row / 8192 | ones row]
    #   yT:  [128, 8*C] bf16      tile-major like xh/xl, gated y
    xh_d = nc.dram_tensor("xh", [128, 8 * C], f8, kind="ExternalInput").ap()
    xl_d = nc.dram_tensor("xl", [128, 8 * C], f8, kind="ExternalInput").ap()
    w1_d = nc.dram_tensor("w1", [8, 128, 2, 8, 128], f8, kind="ExternalInput").ap()
    w2_d = nc.dram_tensor("w2", [8, 128, 2, 8, 128], f8, kind="ExternalInput").ap()
    bb_d = nc.dram_tensor("bb", [128, 16], f32, kind="ExternalInput").ap()
    gb_d = nc.dram_tensor("gb", [128, C], f32, kind="ExternalInput").ap()
    yT_d = nc.dram_tensor("yT", [128, 8 * C], bf16, kind="ExternalOutput").ap()

    with tile.TileContext(nc) as tc, ExitStack() as ctx:
        wpool = ctx.enter_context(tc.tile_pool(name="w", bufs=1))
        cpool = ctx.enter_context(tc.tile_pool(name="const", bufs=1))
        xpool = ctx.enter_context(tc.tile_pool(name="x", bufs=2))
        hxpool = ctx.enter_context(tc.tile_pool(name="hx", bufs=3))
        hpool = ctx.enter_context(tc.tile_pool(name="h", bufs=2))
        ypool = ctx.enter_context(tc.tile_pool(name="y", bufs=2))
        gpool = ctx.enter_context(tc.tile_pool(name="g", bufs=2))
        php = ctx.enter_context(tc.tile_pool(name="ph", bufs=3, space="PSUM"))
        pyp = ctx.enter_context(tc.tile_pool(name="py", bufs=4, space="PSUM"))
        pwp = ctx.enter_context(tc.tile_pool(name="pw", bufs=1, space="PSUM"))

        # PE warm-up fed by a small memset (no DMA dependency): dummy bf16
        # matmuls keep the PE busy from ~1us so the cost-model p-state
        # reaches full speed right as the first real matmuls arrive. A
        # dummy relu warms the ACT function table (1.3us load) in the
        # shadow of the DMA ramp.
        wsrc = cpool.tile([1, 256], bf16, tag="wsrc")
        nc.vector.memset(wsrc[:], 1.0)
        dummy = cpool.tile([1, 128], bf16, tag="dummy")
        nc.scalar.activation(dummy[:], wsrc[0:1, 0:128],
                             mybir.ActivationFunctionType.Relu,
                             bias=wsrc[0:1, 0:1], scale=1.0)
        warm = pwp.tile([128, 256], f32, tag="warm")
        for _ in range(18):
            nc.tensor.matmul(warm[:], wsrc[:, 0:128], wsrc[:, 0:256],
                             start=True, stop=True)

        # DMA emission in consumption order (transfers serialize on the
        # DMA bus and dispatches on HWDGE at ~650ns each): w1 strip 0 and
        # tile-0 x first, consts, remaining w1 strips, tile-1 x, w2 strips
        TT0 = tok_tiles[0]
        w1_sb = [None] * 8
        w1_first = wpool.tile([128, 2, 8, 128], f8, tag="w1_0")
        nc.sync.dma_start(w1_first[:], w1_d[0])
        w1_sb[0] = w1_first
        xh0 = xpool.tile([128, 8, TT0], f8, tag="xh")
        nc.sync.dma_start(xh0[:], xh_d[:, 0:8 * TT0])
        xl0 = xpool.tile([128, 8, TT0], f8, tag="xl")
        nc.sync.dma_start(xl0[:], xl_d[:, 0:8 * TT0])
        bb_sb = cpool.tile([128, 16], f32, tag="bb")
        nc.sync.dma_start(bb_sb[:], bb_d[:])
        for j in range(1, 8):
            w1_strip = wpool.tile([128, 2, 8, 128], f8, tag=f"w1_{j}")
            nc.sync.dma_start(w1_strip[:], w1_d[j])
            w1_sb[j] = w1_strip

        g_tiles = [None] * len(tok_tiles)
        x_tiles = [None] * len(tok_tiles)
        x_tiles[0] = (xh0, xl0)
        if len(tok_tiles) > 1:
            TT1 = tok_tiles[1]
            sl1 = slice(8 * TT0, 8 * (TT0 + TT1))
            xh1 = xpool.tile([128, 8, TT1], f8, tag="xh")
            nc.sync.dma_start(xh1[:], xh_d[:, sl1])
            xl1 = xpool.tile([128, 8, TT1], f8, tag="xl")
            nc.sync.dma_start(xl1[:], xl_d[:, sl1])
            x_tiles[1] = (xh1, xl1)
        gb0 = gpool.tile([128, TT0], f32, tag="gbc")
        nc.sync.dma_start(gb0[:], gb_d[:, 0:TT0])
        g_tiles[0] = gb0
        if len(tok_tiles) > 1:
            gb1 = gpool.tile([128, tok_tiles[1]], f32, tag="gbc")
            nc.sync.dma_start(gb1[:], gb_d[:, TT0:TT0 + tok_tiles[1]])
            g_tiles[1] = gb1
        w2_sb = [None] * 8
        for o in range(8):
            w2_strip = wpool.tile([128, 2, 8, 128], f8, tag=f"w2_{o}")
            nc.sync.dma_start(w2_strip[:], w2_d[o])
            w2_sb[o] = w2_strip

        tile_pos = np.cumsum([0] + tok_tiles).tolist()
        ntile = len(tok_tiles)
        h_tiles = [None] * ntile
        assert len(g_tiles) == ntile

        def emit_l1(t):
            """Layer 1 + gate broadcast of tile t; leaves h8/hl8 + g_bc."""
            TT = tok_tiles[t]

            # prefetch x for tile t+1 (tiles 0 and 1 issued upfront)
            nt = t + 1
            if nt < ntile and x_tiles[nt] is None:
                TTn = tok_tiles[nt]
                nsl = slice(8 * tile_pos[nt], 8 * (tile_pos[nt] + TTn))
                xh_p = xpool.tile([128, 8, TTn], f8, tag="xh")
                nc.sync.dma_start(xh_p[:], xh_d[:, nsl])
                xl_p = xpool.tile([128, 8, TTn], f8, tag="xl")
                nc.sync.dma_start(xl_p[:], xl_d[:, nsl])
                x_tiles[nt] = (xh_p, xl_p)
                gb_p = gpool.tile([128, TTn], f32, tag="gbc")
                nc.sync.dma_start(gb_p[:],
                                  gb_d[:, tile_pos[nt]:tile_pos[nt] + TTn])
                g_tiles[nt] = gb_p

            xh_sb, xl_sb = x_tiles[t]

            # layer 1: 64*h^T[j] = relu(64*sum_d W1[d,j]^T x^T[d] + 64*b1[j])
            # 3 fp8 DoubleRow streams: xh@W1h + xl@W1h + xh@W1l
            h8 = [hpool.tile([128, 2, TT], f8, tag=f"h8_{q}", name=f"h8_{q}")
                  for q in range(4)]
            hl8 = [hpool.tile([128, 2, TT], f8, tag=f"hl8_{q}", name=f"hl8_{q}")
                   for q in range(4)]
            for j in range(8):
                # full-bank PSUM tile: a start=True matmul clears the whole
                # 2KB zero region, so sub-bank tiles must not share banks
                ph_t = php.tile([128, 512], f32, tag="ph")
                ph = ph_t[:, 0:TT]
                n = 0
                for v, xs in ((0, xh_sb), (1, xh_sb), (0, xl_sb)):
                    for q in range(4):
                        nc.tensor.matmul(ph,
                                         w1_sb[j][:, v, 2 * q:2 * q + 2, :],
                                         xs[:, 2 * q:2 * q + 2, :],
                                         start=(n == 0), stop=(n == 11),
                                         perf_mode=DR)
                        n += 1
                hx32 = hxpool.tile([128, TT], f32, tag="hx32")
                nc.scalar.activation(hx32[:], ph, RELU,
                                     bias=bb_sb[:, j:j + 1], scale=1.0)
                h8s = h8[j // 2][:, j % 2, :]
                nc.scalar.activation(h8s, hx32[:], COPY)
                nc.vector.scalar_tensor_tensor(hl8[j // 2][:, j % 2, :],
                                               h8s, -1.0, hx32[:],
                                               op0=MULT, op1=ADD)
            h_tiles[t] = (h8, hl8)

        def emit_l2(t):
            """Gate broadcast + layer 2 + output DMA of tile t."""
            TT = tok_tiles[t]
            h8, hl8 = h_tiles[t]
            last = t == ntile - 1

            g_bc = g_tiles[t]

            # layer 2 + gate: y^T[o] = (sum_j W2[j,o]^T h^T[j] + b2[o]) * g
            # 3 fp8 DoubleRow streams: h8@W2h + hl8@W2h + h8@W2l
            ybig = ypool.tile([128, 8, TT], bf16, tag="y")
            for o in range(8):
                py_t = pyp.tile([128, 512], f32, tag="py")
                py = py_t[:, 0:TT]
                n = 0
                for v, hs in ((0, h8), (0, hl8), (1, h8)):
                    for q in range(4):
                        nc.tensor.matmul(py,
                                         w2_sb[o][:, v, 2 * q:2 * q + 2, :],
                                         hs[q][:],
                                         start=(n == 0), stop=(n == 11),
                                         perf_mode=DR)
                        n += 1
                nc.vector.scalar_tensor_tensor(ybig[:, o, :], py,
                                               bb_sb[:, 8 + o:9 + o],
                                               g_bc[:], op0=ADD, op1=MULT)
                # dispatch output pieces as their o-groups complete so the
                # transfers drain off the bus before the program tail
                base = 8 * tile_pos[t]
                if o == 3:
                    nc.sync.dma_start(yT_d[:, base:base + 4 * TT],
                                      ybig[:, 0:4, :])
                if last and o == 6:
                    nc.sync.dma_start(yT_d[:, base + 4 * TT:base + 7 * TT],
                                      ybig[:, 4:7, :])
            base = 8 * tile_pos[t]
            if last:
                nc.sync.dma_start(yT_d[:, base + 7 * TT:base + 8 * TT],
                                  ybig[:, 7:8, :])
            else:
                nc.sync.dma_start(yT_d[:, base + 4 * TT:base + 8 * TT],
                                  ybig[:, 4:8, :])

        # software pipeline: layer 1 of tile t+1 runs (on PE) before layer
        # 2 of tile t, so the h-eviction chain (ACT relu -> ACT fp8 cast ->
        # DVE residual) of tile t hides under tile t+1's layer-1 matmuls.
        emit_l1(0)
        for t in range(1, ntile):
            emit_l1(t)
            emit_l2(t - 1)
        emit_l2(ntile - 1)

    nc.compile()
    return nc


def _route(x, Wg, bg):
    """Host gating: fp32 softmax + top-2, matching jax.lax.top_k semantics."""
    logits = x @ Wg + bg
    m = logits.max(axis=1, keepdims=True)
    e = np.exp(logits - m)
    gates = e / e.sum(axis=1, keepdims=True)
    # stable argsort on negated values = ties broken by lower index (jax)
    order = np.argsort(-gates, axis=1, kind="stable")[:, :TOP_K]
    return gates, order


def _q8(a):
    import ml_dtypes
    return np.asarray(a).astype(ml_dtypes.float8_e4m3)


def _pack_w(W, scale):
    """[1024,1024] -> [8, 128, 2, 8, 128] hi/lo fp8 strips.

    out[s, p, v, d, r] = q_v(scale * W[d*128+p, s*128+r])
    """
    Ws = (W * scale).astype(np.float32)
    Wh = _q8(Ws)
    Wl = _q8(Ws - Wh.astype(np.float32))
    packs = []
    for Wv in (Wh, Wl):
        # [d, p, s, r] -> [s, p, d, r]
        packs.append(Wv.reshape(8, 128, 8, 128).transpose(2, 1, 0, 3))
    # -> [s, p, v, d, r]
    return np.ascontiguousarray(np.stack(packs, axis=2))


def _pack_x_tiles(xq, toks, tok_tiles, C):
    """Gather + transpose + tile-major pack: [128, 8*C] fp8."""
    out = np.zeros((128, 8 * C), dtype=xq.dtype)
    ne = len(toks)
    pos = 0
    for TT in tok_tiles:
        take = toks[pos:pos + TT]
        if len(take):
            # [nt, 1024] -> [128, 8, nt]
            seg = xq[take].T.reshape(8, 128, len(take)).transpose(1, 0, 2)
            blk = out[:, 8 * pos:8 * (pos + TT)].reshape(128, 8, TT)
            blk[:, :, :len(take)] = seg
        pos += TT
    return out


def _make_in_maps(x, W1, b1, W2, b2, gates, order, tok_lists, C, tok_tiles):
    xh_full = _q8(x)
    xl_full = _q8(x - xh_full.astype(np.float32))
    in_maps = []
    for e in range(NUM_EXPERTS):
        toks = tok_lists[e]
        g_e = np.zeros(C, dtype=np.float32)
        g_e[:len(toks)] = gates[toks, e] / (S1 * S2)
        in_maps.append({
            "xh": _pack_x_tiles(xh_full, toks, tok_tiles, C),
            "xl": _pack_x_tiles(xl_full, toks, tok_tiles, C),
            "w1": _pack_w(W1[e], S1),
            "w2": _pack_w(W2[e], S2),
            "bb": np.ascontiguousarray(np.concatenate(
                [(S1 * b1[e]).reshape(8, 128).T,
                 (S1 * S2 * b2[e]).reshape(8, 128).T], axis=1)),
            "gb": np.ascontiguousarray(
                np.broadcast_to(g_e, (128, C))),
        })
    return in_maps


def kernel(x, W1, b1, W2, b2, Wg, bg):
    from concourse import bass_utils

    x = np.ascontiguousarray(np.asarray(x, dtype=np.float32))
    W1 = np.asarray(W1, dtype=np.float32)
    b1 = np.asarray(b1, dtype=np.float32)
    W2 = np.asarray(W2, dtype=np.float32)
    b2 = np.asarray(b2, dtype=np.float32)
    Wg = np.asarray(Wg, dtype=np.float32)
    bg = np.asarray(bg, dtype=np.float32)
    n = x.shape[0]

    gates, order = _route(x, Wg, bg)
    tok_lists = [np.where((order == e).any(axis=1))[0] for e in range(NUM_EXPERTS)]
    max_load = max(len(t) for t in tok_lists)
    C, tok_tiles = _plan_tiles(max_load)

    key = (C, tuple(tok_tiles))
    if key not in _prog_cache:
        _prog_cache[key] = _build_program((C, tok_tiles))
    nc = _prog_cache[key]

    in_maps = _make_in_maps(x, W1, b1, W2, b2, gates, order, tok_lists, C,
                            tok_tiles)
    res = bass_utils.run_bass_kernel_spmd(nc, in_maps, list(range(NUM_EXPERTS)))
    # yT result: tile-major [128, 8*C] bf16 -> [E, 128, 8, C] f32
    yT_all = np.empty((NUM_EXPERTS, 128, 8, C), dtype=np.float32)
    for e in range(NUM_EXPERTS):
        flat = res.results[e]["yT"].astype(np.float32)
        pos = 0
        for TT in tok_tiles:
            yT_all[e, :, :, pos:pos + TT] = (
                flat[:, 8 * pos:8 * (pos + TT)].reshape(128, 8, TT))
            pos += TT

    # scatter-add the two expert contributions per token (already gated)
    slot = np.zeros((NUM_EXPERTS, n), dtype=np.int64)
    for e in range(NUM_EXPERTS):
        slot[e, tok_lists[e]] = np.arange(len(tok_lists[e]))
    rows = np.arange(n)
    out = np.zeros((n, D), dtype=np.float32)
    for k in range(TOP_K):
        ek = order[:, k]
        picked = yT_all[ek, :, :, slot[ek, rows]]   # [n, 128, 8]
        out += picked.transpose(0, 2, 1).reshape(n, D)
    return out


# revision 26
# speedup vs baseline: 1.1114x; 1.1073x over previous
"""Trainium2 Bass kernel for an 8-expert top-2 MoE layer.

Strategy (expert-parallel, per the sharding hint): the host computes the
tiny gating matmul + softmax + top-2 routing, gathers each expert's
assigned tokens, and ships one expert per NeuronCore. Each core runs the
heavy 2-layer MLP for its expert over its assigned tokens, applies the
gate weights on-device, and the host scatter-adds the two expert
contributions per token.

The MLP matmuls run as fp8(e4m3) DoubleRow pair-matmuls (each
instruction contracts K=256 = 2 k-tiles at half-rate-per-row), with
*residual compensation* to keep accuracy: every operand A is shipped as
a hi/lo pair (A_hi = fp8(A), A_lo = fp8(A - A_hi), same scale), and each
1024-contraction runs three streams

    A_hi @ W_hi  +  A_lo @ W_hi  +  A_hi @ W_lo

which costs 12 pair-matmuls per 128-wide output group (vs 8 full-rate
matmuls for f32r) -> 0.75 cycles/row/layer equivalent, and leaves only
residual-of-residual error (~3e-3 max-rel, gate is 2e-2).

Scales are powers of two folded into host-prepped constants:
  W1 is shipped as fp8(64*W1), so PSUM1 = 64*(x@W1);
  h is evicted as relu(PSUM1 + 64*b1) = 64*h (max ~206 < 240 = e4m3 max)
  via one ACT relu (bias AP), then cast to fp8 (hi) on ACT and the
  residual (lo) computed on DVE;
  W2 is shipped as fp8(128*W2), so PSUM2 = 8192*(h@W2), and the y
  eviction folds b2*8192 and gate/8192 into one (psum + b2') * gate'
  DVE op; y ships bf16.

Schedule: token tiles (<=512, one fp32 PSUM bank) are software-
pipelined as L1(0) L1(1) L2(0) L1(2) L2(1) ... so the h-eviction chain
(ACT relu -> ACT fp8 cast -> DVE residual) of tile t hides under tile
t+1's layer-1 matmuls. x/y use a tile-major DRAM layout (each tile's 8
d-rows contiguous per partition -> >=2KB DMA runs at full model
bandwidth, 128 descriptors per transfer). The first tile is ~296 tokens
so its x lands early but its groups still consume weight strips no
faster than the (HWDGE-serialized) strips arrive. Warm-up matmuls off a
memset tile keep the PE p-state ramping from ~1us with no DMA
dependency; the last tile's output DMA is split in halves so only a
quarter of it trails the final matmul.
"""

import numpy as np

NUM_EXPERTS = 8
TOP_K = 2
D = 1024
S1 = 64.0     # W1/h scale
S2 = 128.0    # W2 scale (gate folds 1/(S1*S2))

_prog_cache = {}


def _plan_tiles(max_load):
    """Token-tile sizes covering max_load: a ~296-token first tile (early
    x arrival without starving on weight strips), then 512s, then a tail
    rounded to a multiple of 8 (fp32r gate matmul ISA restriction)."""
    r8 = lambda v: -(-v // 8) * 8
    tiles = []
    rest = max_load
    for first in (296, 384):
        if rest <= 0:
            break
        take = min(first, rest)
        tiles.append(r8(take))
        rest -= take
    n512, rem = divmod(max(rest, 0), 512)
    tiles += [512] * n512
    if rem:
        tiles.append(r8(rem))
    return sum(tiles), tiles


def _build_program(tile_plan):
    """Build the per-core Bass program: one expert's MLP over C tokens."""
    from contextlib import ExitStack

    import concourse.tile as tile
    from concourse import bacc, mybir

    f32 = mybir.dt.float32
    f32r = mybir.dt.float32r
    f8 = mybir.dt.float8e4
    bf16 = mybir.dt.bfloat16
    DR = mybir.MatmulPerfMode.DoubleRow
    ADD = mybir.AluOpType.add
    MULT = mybir.AluOpType.mult
    RELU = mybir.ActivationFunctionType.Relu
    COPY = mybir.ActivationFunctionType.Copy

    C, tok_tiles = tile_plan

    nc = bacc.Bacc("TRN2", target_bir_lowering=False, debug=False,
                   num_devices=NUM_EXPERTS)

    # host-packed layouts (see _make_in_maps), all e4m3 except consts:
    #   xh/xl: [128, 8*C] tile-major: cols [8*pos_t + d*TT_t + c]
    #          = q(x_gathered[pos_t + c, d*128 + p]) hi/lo
    #   w1:  [8, 128, 2, 8, 128]  w1[j, p, v, d, r] = q(64*W1[d*128+p, j*128+r])
    #   w2:  [8, 128, 2, 8, 128]  w2[o, p, v, j, r] = q(128*W2[j*128+p, o*128+r])
    #   bb:  [128, 16] f32        [64*b1 | 8192*b2] per-partition
    #   go:  [1, C+128] f32r      [gate row / 8192 | ones row]
    #   yT:  [128, 8*C] bf16      tile-major like xh/xl, gated y
    xh_d = nc.dram_tensor("xh", [128, 8 * C], f8, kind="ExternalInput").ap()
    xl_d = nc.dram_tensor("xl", [128, 8 * C], f8, kind="ExternalInput").ap()
    w1_d = nc.dram_tensor("w1", [8, 128, 2, 8, 128], f8, kind="ExternalInput").ap()
    w2_d = nc.dram_tensor("w2", [8, 128, 2, 8, 128], f8, kind="ExternalInput").ap()
    bb_d = nc.dram_tensor("bb", [128, 16], f32, kind="ExternalInput").ap()
    gb_d = nc.dram_tensor("gb", [128, C], f32, kind="ExternalInput").ap()
    yT_d = nc.dram_tensor("yT", [128, 8 * C], bf16, kind="ExternalOutput").ap()

    with tile.TileContext(nc) as tc, ExitStack() as ctx:
        wpool = ctx.enter_context(tc.tile_pool(name="w", bufs=1))
        cpool = ctx.enter_context(tc.tile_pool(name="const", bufs=1))
        xpool = ctx.enter_context(tc.tile_pool(name="x", bufs=2))
        hxpool = ctx.enter_context(tc.tile_pool(name="hx", bufs=3))
        hpool = ctx.enter_context(tc.tile_pool(name="h", bufs=2))
        ypool = ctx.enter_context(tc.tile_pool(name="y", bufs=2))
        gpool = ctx.enter_context(tc.tile_pool(name="g", bufs=2))
        php = ctx.enter_context(tc.tile_pool(name="ph", bufs=3, space="PSUM"))
        pyp = ctx.enter_context(tc.tile_pool(name="py", bufs=4, space="PSUM"))
        pwp = ctx.enter_context(tc.tile_pool(name="pw", bufs=1, space="PSUM"))

        # PE warm-up fed by a small memset (no DMA dependency): dummy bf16
        # matmuls keep the PE busy from ~1us so the cost-model p-state
        # reaches full speed right as the first real matmuls arrive. A
        # dummy relu warms the ACT function table (1.3us load) in the
        # shadow of the DMA ramp.
        wsrc = cpool.tile([1, 256], bf16, tag="wsrc")
        nc.vector.memset(wsrc[:], 1.0)
        dummy = cpool.tile([1, 128], bf16, tag="dummy")
        nc.scalar.activation(dummy[:], wsrc[0:1, 0:128],
                             mybir.ActivationFunctionType.Relu,
                             bias=wsrc[0:1, 0:1], scale=1.0)
        warm = pwp.tile([128, 256], f32, tag="warm")
        for _ in range(18):
            nc.tensor.matmul(warm[:], wsrc[:, 0:128], wsrc[:, 0:256],
                             start=True, stop=True)

        # DMA emission in consumption order (transfers serialize on the
        # DMA bus and dispatches on HWDGE at ~650ns each): w1 strip 0 and
        # tile-0 x first, consts, remaining w1 strips, tile-1 x, w2 strips
        TT0 = tok_tiles[0]
        w1_sb = [None] * 8
        w1_first = wpool.tile([128, 2, 8, 128], f8, tag="w1_0")
        nc.sync.dma_start(w1_first[:], w1_d[0])
        w1_sb[0] = w1_first
        xh0 = xpool.tile([128, 8, TT0], f8, tag="xh")
        nc.sync.dma_start(xh0[:], xh_d[:, 0:8 * TT0])
        xl0 = xpool.tile([128, 8, TT0], f8, tag="xl")
        nc.sync.dma_start(xl0[:], xl_d[:, 0:8 * TT0])
        bb_sb = cpool.tile([128, 16], f32, tag="bb")
        nc.sync.dma_start(bb_sb[:], bb_d[:])
        for j in range(1, 8):
            w1_strip = wpool.tile([128, 2, 8, 128], f8, tag=f"w1_{j}")
            nc.sync.dma_start(w1_strip[:], w1_d[j])
            w1_sb[j] = w1_strip

        g_tiles = [None] * len(tok_tiles)
        x_tiles = [None] * len(tok_tiles)
        x_tiles[0] = (xh0, xl0)
        if len(tok_tiles) > 1:
            TT1 = tok_tiles[1]
            sl1 = slice(8 * TT0, 8 * (TT0 + TT1))
            xh1 = xpool.tile([128, 8, TT1], f8, tag="xh")
            nc.sync.dma_start(xh1[:], xh_d[:, sl1])
            xl1 = xpool.tile([128, 8, TT1], f8, tag="xl")
            nc.sync.dma_start(xl1[:], xl_d[:, sl1])
            x_tiles[1] = (xh1, xl1)
        gb0 = gpool.tile([128, TT0], f32, tag="gbc")
        nc.sync.dma_start(gb0[:], gb_d[:, 0:TT0])
        g_tiles[0] = gb0
        if len(tok_tiles) > 1:
            gb1 = gpool.tile([128, tok_tiles[1]], f32, tag="gbc")
            nc.sync.dma_start(gb1[:], gb_d[:, TT0:TT0 + tok_tiles[1]])
            g_tiles[1] = gb1
        w2_sb = [None] * 8
        for o in range(8):
            w2_strip = wpool.tile([128, 2, 8, 128], f8, tag=f"w2_{o}")
            nc.sync.dma_start(w2_strip[:], w2_d[o])
            w2_sb[o] = w2_strip

        tile_pos = np.cumsum([0] + tok_tiles).tolist()
        ntile = len(tok_tiles)
        h_tiles = [None] * ntile
        assert len(g_tiles) == ntile

        def emit_l1(t):
            """Layer 1 + gate broadcast of tile t; leaves h8/hl8 + g_bc."""
            TT = tok_tiles[t]

            # prefetch x for tile t+1 (tiles 0 and 1 issued upfront)
            nt = t + 1
            if nt < ntile and x_tiles[nt] is None:
                TTn = tok_tiles[nt]
                nsl = slice(8 * tile_pos[nt], 8 * (tile_pos[nt] + TTn))
                xh_p = xpool.tile([128, 8, TTn], f8, tag="xh")
                nc.sync.dma_start(xh_p[:], xh_d[:, nsl])
                xl_p = xpool.tile([128, 8, TTn], f8, tag="xl")
                nc.sync.dma_start(xl_p[:], xl_d[:, nsl])
                x_tiles[nt] = (xh_p, xl_p)
                gb_p = gpool.tile([128, TTn], f32, tag="gbc")
                nc.sync.dma_start(gb_p[:],
                                  gb_d[:, tile_pos[nt]:tile_pos[nt] + TTn])
                g_tiles[nt] = gb_p

            xh_sb, xl_sb = x_tiles[t]

            # layer 1: 64*h^T[j] = relu(64*sum_d W1[d,j]^T x^T[d] + 64*b1[j])
            # 3 fp8 DoubleRow streams: xh@W1h + xl@W1h + xh@W1l
            h8 = [hpool.tile([128, 2, TT], f8, tag=f"h8_{q}", name=f"h8_{q}")
                  for q in range(4)]
            hl8 = [hpool.tile([128, 2, TT], f8, tag=f"hl8_{q}", name=f"hl8_{q}")
                   for q in range(4)]
            # correction streams cover 3 of 4 k-pairs: each dropped pair
            # costs sqrt(1/4) of that source's error (measured total
            # 1.62e-2 relmax vs the 2e-2 gate) and saves 1/24 of PE time
            L1_STREAMS = (((0, 0), (0, 1), (0, 2), (0, 3)),   # xh @ W1h
                          ((1, 1), (1, 2), (1, 3)),           # xh @ W1l
                          ((0, 0), (0, 1), (0, 2)))           # xl @ W1h
            nmm = sum(len(s) for s in L1_STREAMS)
            for j in range(8):
                # full-bank PSUM tile: a start=True matmul clears the whole
                # 2KB zero region, so sub-bank tiles must not share banks
                ph_t = php.tile([128, 512], f32, tag="ph")
                ph = ph_t[:, 0:TT]
                n = 0
                for si, stream in enumerate(L1_STREAMS):
                    xs = xh_sb if si < 2 else xl_sb
                    for v, q in stream:
                        nc.tensor.matmul(ph,
                                         w1_sb[j][:, v, 2 * q:2 * q + 2, :],
                                         xs[:, 2 * q:2 * q + 2, :],
                                         start=(n == 0), stop=(n == nmm - 1),
                                         perf_mode=DR)
                        n += 1
                hx32 = hxpool.tile([128, TT], f32, tag="hx32")
                nc.scalar.activation(hx32[:], ph, RELU,
                                     bias=bb_sb[:, j:j + 1], scale=1.0)
                h8s = h8[j // 2][:, j % 2, :]
                nc.scalar.activation(h8s, hx32[:], COPY)
                nc.vector.scalar_tensor_tensor(hl8[j // 2][:, j % 2, :],
                                               h8s, -1.0, hx32[:],
                                               op0=MULT, op1=ADD)
            h_tiles[t] = (h8, hl8)

        def emit_l2(t):
            """Gate broadcast + layer 2 + output DMA of tile t."""
            TT = tok_tiles[t]
            h8, hl8 = h_tiles[t]
            last = t == ntile - 1

            g_bc = g_tiles[t]

            # layer 2 + gate: y^T[o] = (sum_j W2[j,o]^T h^T[j] + b2[o]) * g
            # 3 fp8 DoubleRow streams: h8@W2h + hl8@W2h + h8@W2l
            ybig = ypool.tile([128, 8, TT], bf16, tag="y")
            L2_STREAMS = (((0, h8, 0), (0, h8, 1), (0, h8, 2), (0, h8, 3)),
                          ((0, hl8, 0), (0, hl8, 1), (0, hl8, 2), (0, hl8, 3)),
                          ((1, h8, 1), (1, h8, 2), (1, h8, 3)))
            nmm2 = sum(len(s) for s in L2_STREAMS)
            for o in range(8):
                py_t = pyp.tile([128, 512], f32, tag="py")
                py = py_t[:, 0:TT]
                n = 0
                for stream in L2_STREAMS:
                    for v, hs, q in stream:
                        nc.tensor.matmul(py,
                                         w2_sb[o][:, v, 2 * q:2 * q + 2, :],
                                         hs[q][:],
                                         start=(n == 0), stop=(n == nmm2 - 1),
                                         perf_mode=DR)
                        n += 1
                nc.vector.scalar_tensor_tensor(ybig[:, o, :], py,
                                               bb_sb[:, 8 + o:9 + o],
                                               g_bc[:], op0=ADD, op1=MULT)
                # dispatch output pieces as their o-groups complete so the
                # transfers drain off the bus before the program tail
                base = 8 * tile_pos[t]
                if o == 3:
                    nc.sync.dma_start(yT_d[:, base:base + 4 * TT],
                                      ybig[:, 0:4, :])
                if last and o == 6:
                    nc.sync.dma_start(yT_d[:, base + 4 * TT:base + 7 * TT],
                                      ybig[:, 4:7, :])
            base = 8 * tile_pos[t]
            if last:
                nc.sync.dma_start(yT_d[:, base + 7 * TT:base + 8 * TT],
                                  ybig[:, 7:8, :])
            else:
                nc.sync.dma_start(yT_d[:, base + 4 * TT:base + 8 * TT],
                                  ybig[:, 4:8, :])

        # software pipeline: layer 1 of tile t+1 runs (on PE) before layer
        # 2 of tile t, so the h-eviction chain (ACT relu -> ACT fp8 cast ->
        # DVE residual) of tile t hides under tile t+1's layer-1 matmuls.
        emit_l1(0)
        for t in range(1, ntile):
            emit_l1(t)
            emit_l2(t - 1)
        emit_l2(ntile - 1)

    nc.compile()
    return nc


def _route(x, Wg, bg):
    """Host gating: fp32 softmax + top-2, matching jax.lax.top_k semantics."""
    logits = x @ Wg + bg
    m = logits.max(axis=1, keepdims=True)
    e = np.exp(logits - m)
    gates = e / e.sum(axis=1, keepdims=True)
    # stable argsort on negated values = ties broken by lower index (jax)
    order = np.argsort(-gates, axis=1, kind="stable")[:, :TOP_K]
    return gates, order


def _q8(a):
    import ml_dtypes
    return np.asarray(a).astype(ml_dtypes.float8_e4m3)


def _pack_w(W, scale):
    """[1024,1024] -> [8, 128, 2, 8, 128] hi/lo fp8 strips.

    out[s, p, v, d, r] = q_v(scale * W[d*128+p, s*128+r])
    """
    Ws = (W * scale).astype(np.float32)
    Wh = _q8(Ws)
    Wl = _q8(Ws - Wh.astype(np.float32))
    packs = []
    for Wv in (Wh, Wl):
        # [d, p, s, r] -> [s, p, d, r]
        packs.append(Wv.reshape(8, 128, 8, 128).transpose(2, 1, 0, 3))
    # -> [s, p, v, d, r]
    return np.ascontiguousarray(np.stack(packs, axis=2))


def _pack_x_tiles(xq, toks, tok_tiles, C):
    """Gather + transpose + tile-major pack: [128, 8*C] fp8."""
    out = np.zeros((128, 8 * C), dtype=xq.dtype)
    ne = len(toks)
    pos = 0
    for TT in tok_tiles:
        take = toks[pos:pos + TT]
        if len(take):
            # [nt, 1024] -> [128, 8, nt]
            seg = xq[take].T.reshape(8, 128, len(take)).transpose(1, 0, 2)
            blk = out[:, 8 * pos:8 * (pos + TT)].reshape(128, 8, TT)
            blk[:, :, :len(take)] = seg
        pos += TT
    return out


def _make_in_maps(x, W1, b1, W2, b2, gates, order, tok_lists, C, tok_tiles):
    xh_full = _q8(x)
    xl_full = _q8(x - xh_full.astype(np.float32))
    in_maps = []
    for e in range(NUM_EXPERTS):
        toks = tok_lists[e]
        g_e = np.zeros(C, dtype=np.float32)
        g_e[:len(toks)] = gates[toks, e] / (S1 * S2)
        in_maps.append({
            "xh": _pack_x_tiles(xh_full, toks, tok_tiles, C),
            "xl": _pack_x_tiles(xl_full, toks, tok_tiles, C),
            "w1": _pack_w(W1[e], S1),
            "w2": _pack_w(W2[e], S2),
            "bb": np.ascontiguousarray(np.concatenate(
                [(S1 * b1[e]).reshape(8, 128).T,
                 (S1 * S2 * b2[e]).reshape(8, 128).T], axis=1)),
            "gb": np.ascontiguousarray(
                np.broadcast_to(g_e, (128, C))),
        })
    return in_maps


def kernel(x, W1, b1, W2, b2, Wg, bg):
    from concourse import bass_utils

    x = np.ascontiguousarray(np.asarray(x, dtype=np.float32))
    W1 = np.asarray(W1, dtype=np.float32)
    b1 = np.asarray(b1, dtype=np.float32)
    W2 = np.asarray(W2, dtype=np.float32)
    b2 = np.asarray(b2, dtype=np.float32)
    Wg = np.asarray(Wg, dtype=np.float32)
    bg = np.asarray(bg, dtype=np.float32)
    n = x.shape[0]

    gates, order = _route(x, Wg, bg)
    tok_lists = [np.where((order == e).any(axis=1))[0] for e in range(NUM_EXPERTS)]
    max_load = max(len(t) for t in tok_lists)
    C, tok_tiles = _plan_tiles(max_load)

    key = (C, tuple(tok_tiles))
    if key not in _prog_cache:
        _prog_cache[key] = _build_program((C, tok_tiles))
    nc = _prog_cache[key]

    in_maps = _make_in_maps(x, W1, b1, W2, b2, gates, order, tok_lists, C,
                            tok_tiles)
    res = bass_utils.run_bass_kernel_spmd(nc, in_maps, list(range(NUM_EXPERTS)))
    # yT result: tile-major [128, 8*C] bf16 -> [E, 128, 8, C] f32
    yT_all = np.empty((NUM_EXPERTS, 128, 8, C), dtype=np.float32)
    for e in range(NUM_EXPERTS):
        flat = res.results[e]["yT"].astype(np.float32)
        pos = 0
        for TT in tok_tiles:
            yT_all[e, :, :, pos:pos + TT] = (
                flat[:, 8 * pos:8 * (pos + TT)].reshape(128, 8, TT))
            pos += TT

    # scatter-add the two expert contributions per token (already gated)
    slot = np.zeros((NUM_EXPERTS, n), dtype=np.int64)
    for e in range(NUM_EXPERTS):
        slot[e, tok_lists[e]] = np.arange(len(tok_lists[e]))
    rows = np.arange(n)
    out = np.zeros((n, D), dtype=np.float32)
    for k in range(TOP_K):
        ek = order[:, k]
        picked = yT_all[ek, :, :, slot[ek, rows]]   # [n, 128, 8]
        out += picked.transpose(0, 2, 1).reshape(n, D)
    return out


# revision 27
# speedup vs baseline: 1.1123x; 1.0008x over previous
"""Trainium2 Bass kernel for an 8-expert top-2 MoE layer.

Strategy (expert-parallel, per the sharding hint): the host computes the
tiny gating matmul + softmax + top-2 routing, gathers each expert's
assigned tokens, and ships one expert per NeuronCore. Each core runs the
heavy 2-layer MLP for its expert over its assigned tokens, applies the
gate weights on-device, and the host scatter-adds the two expert
contributions per token.

The MLP matmuls run as fp8(e4m3) DoubleRow pair-matmuls (each
instruction contracts K=256 = 2 k-tiles at half-rate-per-row), with
*residual compensation* to keep accuracy: every operand A is shipped as
a hi/lo pair (A_hi = fp8(A), A_lo = fp8(A - A_hi), same scale), and each
1024-contraction runs three streams

    A_hi @ W_hi  +  A_lo @ W_hi  +  A_hi @ W_lo

which costs 12 pair-matmuls per 128-wide output group (vs 8 full-rate
matmuls for f32r) -> 0.75 cycles/row/layer equivalent, and leaves only
residual-of-residual error (~3e-3 max-rel, gate is 2e-2).

Scales are powers of two folded into host-prepped constants:
  W1 is shipped as fp8(64*W1), so PSUM1 = 64*(x@W1);
  h is evicted as relu(PSUM1 + 64*b1) = 64*h (max ~206 < 240 = e4m3 max)
  via one ACT relu (bias AP), then cast to fp8 (hi) on ACT and the
  residual (lo) computed on DVE;
  W2 is shipped as fp8(128*W2), so PSUM2 = 8192*(h@W2), and the y
  eviction folds b2*8192 and gate/8192 into one (psum + b2') * gate'
  DVE op; y ships bf16.

Schedule: token tiles (<=512, one fp32 PSUM bank) are software-
pipelined as L1(0) L1(1) L2(0) L1(2) L2(1) ... so the h-eviction chain
(ACT relu -> ACT fp8 cast -> DVE residual) of tile t hides under tile
t+1's layer-1 matmuls. x/y use a tile-major DRAM layout (each tile's 8
d-rows contiguous per partition -> >=2KB DMA runs at full model
bandwidth, 128 descriptors per transfer). The first tile is ~296 tokens
so its x lands early but its groups still consume weight strips no
faster than the (HWDGE-serialized) strips arrive. Warm-up matmuls off a
memset tile keep the PE p-state ramping from ~1us with no DMA
dependency; the last tile's output DMA is split in halves so only a
quarter of it trails the final matmul.
"""

import numpy as np

NUM_EXPERTS = 8
TOP_K = 2
D = 1024
S1 = 64.0     # W1/h scale
S2 = 128.0    # W2 scale (gate folds 1/(S1*S2))

_prog_cache = {}


def _plan_tiles(max_load):
    """Token-tile sizes covering max_load: a ~296-token first tile (early
    x arrival without starving on weight strips), then 512s, then a tail
    rounded to a multiple of 8 (fp32r gate matmul ISA restriction)."""
    r8 = lambda v: -(-v // 8) * 8
    tiles = []
    rest = max_load
    for first in (352, 384):
        if rest <= 0:
            break
        take = min(first, rest)
        tiles.append(r8(take))
        rest -= take
    n512, rem = divmod(max(rest, 0), 512)
    tiles += [512] * n512
    if rem:
        tiles.append(r8(rem))
    return sum(tiles), tiles


def _build_program(tile_plan):
    """Build the per-core Bass program: one expert's MLP over C tokens."""
    from contextlib import ExitStack

    import concourse.tile as tile
    from concourse import bacc, mybir

    f32 = mybir.dt.float32
    f32r = mybir.dt.float32r
    f8 = mybir.dt.float8e4
    bf16 = mybir.dt.bfloat16
    DR = mybir.MatmulPerfMode.DoubleRow
    ADD = mybir.AluOpType.add
    MULT = mybir.AluOpType.mult
    RELU = mybir.ActivationFunctionType.Relu
    COPY = mybir.ActivationFunctionType.Copy

    C, tok_tiles = tile_plan

    nc = bacc.Bacc("TRN2", target_bir_lowering=False, debug=False,
                   num_devices=NUM_EXPERTS)

    # host-packed layouts (see _make_in_maps), all e4m3 except consts:
    #   xh/xl: [128, 8*C] tile-major: cols [8*pos_t + d*TT_t + c]
    #          = q(x_gathered[pos_t + c, d*128 + p]) hi/lo
    #   w1:  [8, 128, 2, 8, 128]  w1[j, p, v, d, r] = q(64*W1[d*128+p, j*128+r])
    #   w2:  [8, 128, 2, 8, 128]  w2[o, p, v, j, r] = q(128*W2[j*128+p, o*128+r])
    #   bb:  [128, 16] f32        [64*b1 | 8192*b2] per-partition
    #   go:  [1, C+128] f32r      [gate row / 8192 | ones row]
    #   yT:  [128, 8*C] bf16      tile-major like xh/xl, gated y
    xh_d = nc.dram_tensor("xh", [128, 8 * C], f8, kind="ExternalInput").ap()
    xl_d = nc.dram_tensor("xl", [128, 8 * C], f8, kind="ExternalInput").ap()
    w1_d = nc.dram_tensor("w1", [8, 128, 2, 8, 128], f8, kind="ExternalInput").ap()
    w2_d = nc.dram_tensor("w2", [8, 128, 2, 8, 128], f8, kind="ExternalInput").ap()
    bb_d = nc.dram_tensor("bb", [128, 16], f32, kind="ExternalInput").ap()
    gb_d = nc.dram_tensor("gb", [128, C], f32, kind="ExternalInput").ap()
    yT_d = nc.dram_tensor("yT", [128, 8 * C], bf16, kind="ExternalOutput").ap()

    with tile.TileContext(nc) as tc, ExitStack() as ctx:
        wpool = ctx.enter_context(tc.tile_pool(name="w", bufs=1))
        cpool = ctx.enter_context(tc.tile_pool(name="const", bufs=1))
        xpool = ctx.enter_context(tc.tile_pool(name="x", bufs=2))
        hxpool = ctx.enter_context(tc.tile_pool(name="hx", bufs=3))
        hpool = ctx.enter_context(tc.tile_pool(name="h", bufs=2))
        ypool = ctx.enter_context(tc.tile_pool(name="y", bufs=2))
        gpool = ctx.enter_context(tc.tile_pool(name="g", bufs=2))
        php = ctx.enter_context(tc.tile_pool(name="ph", bufs=3, space="PSUM"))
        pyp = ctx.enter_context(tc.tile_pool(name="py", bufs=4, space="PSUM"))
        pwp = ctx.enter_context(tc.tile_pool(name="pw", bufs=1, space="PSUM"))

        # PE warm-up fed by a small memset (no DMA dependency): dummy bf16
        # matmuls keep the PE busy from ~1us so the cost-model p-state
        # reaches full speed right as the first real matmuls arrive. A
        # dummy relu warms the ACT function table (1.3us load) in the
        # shadow of the DMA ramp.
        wsrc = cpool.tile([1, 256], bf16, tag="wsrc")
        nc.vector.memset(wsrc[:], 1.0)
        dummy = cpool.tile([1, 128], bf16, tag="dummy")
        nc.scalar.activation(dummy[:], wsrc[0:1, 0:128],
                             mybir.ActivationFunctionType.Relu,
                             bias=wsrc[0:1, 0:1], scale=1.0)
        warm = pwp.tile([128, 256], f32, tag="warm")
        for _ in range(21):
            nc.tensor.matmul(warm[:], wsrc[:, 0:128], wsrc[:, 0:256],
                             start=True, stop=True)

        # DMA emission in consumption order (transfers serialize on the
        # DMA bus and dispatches on HWDGE at ~650ns each): w1 strip 0 and
        # tile-0 x first, consts, remaining w1 strips, tile-1 x, w2 strips
        TT0 = tok_tiles[0]
        w1_sb = [None] * 8
        w1_first = wpool.tile([128, 2, 8, 128], f8, tag="w1_0")
        nc.sync.dma_start(w1_first[:], w1_d[0])
        w1_sb[0] = w1_first
        xh0 = xpool.tile([128, 8, TT0], f8, tag="xh")
        nc.sync.dma_start(xh0[:], xh_d[:, 0:8 * TT0])
        xl0 = xpool.tile([128, 8, TT0], f8, tag="xl")
        nc.sync.dma_start(xl0[:], xl_d[:, 0:8 * TT0])
        bb_sb = cpool.tile([128, 16], f32, tag="bb")
        nc.sync.dma_start(bb_sb[:], bb_d[:])
        for j in range(1, 8):
            w1_strip = wpool.tile([128, 2, 8, 128], f8, tag=f"w1_{j}")
            nc.sync.dma_start(w1_strip[:], w1_d[j])
            w1_sb[j] = w1_strip

        g_tiles = [None] * len(tok_tiles)
        x_tiles = [None] * len(tok_tiles)
        x_tiles[0] = (xh0, xl0)
        if len(tok_tiles) > 1:
            TT1 = tok_tiles[1]
            sl1 = slice(8 * TT0, 8 * (TT0 + TT1))
            xh1 = xpool.tile([128, 8, TT1], f8, tag="xh")
            nc.sync.dma_start(xh1[:], xh_d[:, sl1])
            xl1 = xpool.tile([128, 8, TT1], f8, tag="xl")
            nc.sync.dma_start(xl1[:], xl_d[:, sl1])
            x_tiles[1] = (xh1, xl1)
        gb0 = gpool.tile([128, TT0], f32, tag="gbc")
        nc.sync.dma_start(gb0[:], gb_d[:, 0:TT0])
        g_tiles[0] = gb0
        if len(tok_tiles) > 1:
            gb1 = gpool.tile([128, tok_tiles[1]], f32, tag="gbc")
            nc.sync.dma_start(gb1[:], gb_d[:, TT0:TT0 + tok_tiles[1]])
            g_tiles[1] = gb1
        w2_sb = [None] * 8
        for o in range(8):
            w2_strip = wpool.tile([128, 2, 8, 128], f8, tag=f"w2_{o}")
            nc.sync.dma_start(w2_strip[:], w2_d[o])
            w2_sb[o] = w2_strip

        tile_pos = np.cumsum([0] + tok_tiles).tolist()
        ntile = len(tok_tiles)
        h_tiles = [None] * ntile
        assert len(g_tiles) == ntile

        def emit_l1(t):
            """Layer 1 + gate broadcast of tile t; leaves h8/hl8 + g_bc."""
            TT = tok_tiles[t]

            # prefetch x for tile t+1 (tiles 0 and 1 issued upfront)
            nt = t + 1
            if nt < ntile and x_tiles[nt] is None:
                TTn = tok_tiles[nt]
                nsl = slice(8 * tile_pos[nt], 8 * (tile_pos[nt] + TTn))
                xh_p = xpool.tile([128, 8, TTn], f8, tag="xh")
                nc.sync.dma_start(xh_p[:], xh_d[:, nsl])
                xl_p = xpool.tile([128, 8, TTn], f8, tag="xl")
                nc.sync.dma_start(xl_p[:], xl_d[:, nsl])
                x_tiles[nt] = (xh_p, xl_p)
                gb_p = gpool.tile([128, TTn], f32, tag="gbc")
                nc.sync.dma_start(gb_p[:],
                                  gb_d[:, tile_pos[nt]:tile_pos[nt] + TTn])
                g_tiles[nt] = gb_p

            xh_sb, xl_sb = x_tiles[t]

            # layer 1: 64*h^T[j] = relu(64*sum_d W1[d,j]^T x^T[d] + 64*b1[j])
            # 3 fp8 DoubleRow streams: xh@W1h + xl@W1h + xh@W1l
            h8 = [hpool.tile([128, 2, TT], f8, tag=f"h8_{q}", name=f"h8_{q}")
                  for q in range(4)]
            hl8 = [hpool.tile([128, 2, TT], f8, tag=f"hl8_{q}", name=f"hl8_{q}")
                   for q in range(4)]
            # correction streams cover 3 of 4 k-pairs: each dropped pair
            # costs sqrt(1/4) of that source's error (measured total
            # 1.62e-2 relmax vs the 2e-2 gate) and saves 1/24 of PE time
            L1_STREAMS = (((0, 0), (0, 1), (0, 2), (0, 3)),   # xh @ W1h
                          ((1, 1), (1, 2), (1, 3)),           # xh @ W1l
                          ((0, 0), (0, 1), (0, 2)))           # xl @ W1h
            nmm = sum(len(s) for s in L1_STREAMS)
            for j in range(8):
                # full-bank PSUM tile: a start=True matmul clears the whole
                # 2KB zero region, so sub-bank tiles must not share banks
                ph_t = php.tile([128, 512], f32, tag="ph")
                ph = ph_t[:, 0:TT]
                n = 0
                for si, stream in enumerate(L1_STREAMS):
                    xs = xh_sb if si < 2 else xl_sb
                    for v, q in stream:
                        nc.tensor.matmul(ph,
                                         w1_sb[j][:, v, 2 * q:2 * q + 2, :],
                                         xs[:, 2 * q:2 * q + 2, :],
                                         start=(n == 0), stop=(n == nmm - 1),
                                         perf_mode=DR)
                        n += 1
                hx32 = hxpool.tile([128, TT], f32, tag="hx32")
                nc.scalar.activation(hx32[:], ph, RELU,
                                     bias=bb_sb[:, j:j + 1], scale=1.0)
                h8s = h8[j // 2][:, j % 2, :]
                nc.scalar.activation(h8s, hx32[:], COPY)
                nc.vector.scalar_tensor_tensor(hl8[j // 2][:, j % 2, :],
                                               h8s, -1.0, hx32[:],
                                               op0=MULT, op1=ADD)
            h_tiles[t] = (h8, hl8)

        def emit_l2(t):
            """Gate broadcast + layer 2 + output DMA of tile t."""
            TT = tok_tiles[t]
            h8, hl8 = h_tiles[t]
            last = t == ntile - 1

            g_bc = g_tiles[t]

            # layer 2 + gate: y^T[o] = (sum_j W2[j,o]^T h^T[j] + b2[o]) * g
            # 3 fp8 DoubleRow streams: h8@W2h + hl8@W2h + h8@W2l
            ybig = ypool.tile([128, 8, TT], bf16, tag="y")
            L2_STREAMS = (((0, h8, 0), (0, h8, 1), (0, h8, 2), (0, h8, 3)),
                          ((0, hl8, 0), (0, hl8, 1), (0, hl8, 2), (0, hl8, 3)),
                          ((1, h8, 1), (1, h8, 2), (1, h8, 3)))
            nmm2 = sum(len(s) for s in L2_STREAMS)
            for o in range(8):
                py_t = pyp.tile([128, 512], f32, tag="py")
                py = py_t[:, 0:TT]
                n = 0
                for stream in L2_STREAMS:
                    for v, hs, q in stream:
                        nc.tensor.matmul(py,
                                         w2_sb[o][:, v, 2 * q:2 * q + 2, :],
                                         hs[q][:],
                                         start=(n == 0), stop=(n == nmm2 - 1),
                                         perf_mode=DR)
                        n += 1
                nc.vector.scalar_tensor_tensor(ybig[:, o, :], py,
                                               bb_sb[:, 8 + o:9 + o],
                                               g_bc[:], op0=ADD, op1=MULT)
                # dispatch output pieces as their o-groups complete so the
                # transfers drain off the bus before the program tail
                base = 8 * tile_pos[t]
                if o == 3:
                    nc.sync.dma_start(yT_d[:, base:base + 4 * TT],
                                      ybig[:, 0:4, :])
                if last and o == 6:
                    nc.sync.dma_start(yT_d[:, base + 4 * TT:base + 7 * TT],
                                      ybig[:, 4:7, :])
            base = 8 * tile_pos[t]
            if last:
                nc.sync.dma_start(yT_d[:, base + 7 * TT:base + 8 * TT],
                                  ybig[:, 7:8, :])
            else:
                nc.sync.dma_start(yT_d[:, base + 4 * TT:base + 8 * TT],
                                  ybig[:, 4:8, :])

        # software pipeline: layer 1 of tile t+1 runs (on PE) before layer
        # 2 of tile t, so the h-eviction chain (ACT relu -> ACT fp8 cast ->
        # DVE residual) of tile t hides under tile t+1's layer-1 matmuls.
        emit_l1(0)
        for t in range(1, ntile):
            emit_l1(t)
            emit_l2(t - 1)
        emit_l2(ntile - 1)

    nc.compile()
    return nc


def _route(x, Wg, bg):
    """Host gating: fp32 softmax + top-2, matching jax.lax.top_k semantics."""
    logits = x @ Wg + bg
    m = logits.max(axis=1, keepdims=True)
    e = np.exp(logits - m)
    gates = e / e.sum(axis=1, keepdims=True)
    # stable argsort on negated values = ties broken by lower index (jax)
    order = np.argsort(-gates, axis=1, kind="stable")[:, :TOP_K]
    return gates, order


def _q8(a):
    import ml_dtypes
    return np.asarray(a).astype(ml_dtypes.float8_e4m3)


def _pack_w(W, scale):
    """[1024,1024] -> [8, 128, 2, 8, 128] hi/lo fp8 strips.

    out[s, p, v, d, r] = q_v(scale * W[d*128+p, s*128+r])
    """
    Ws = (W * scale).astype(np.float32)
    Wh = _q8(Ws)
    Wl = _q8(Ws - Wh.astype(np.float32))
    packs = []
    for Wv in (Wh, Wl):
        # [d, p, s, r] -> [s, p, d, r]
        packs.append(Wv.reshape(8, 128, 8, 128).transpose(2, 1, 0, 3))
    # -> [s, p, v, d, r]
    return np.ascontiguousarray(np.stack(packs, axis=2))


def _pack_x_tiles(xq, toks, tok_tiles, C):
    """Gather + transpose + tile-major pack: [128, 8*C] fp8."""
    out = np.zeros((128, 8 * C), dtype=xq.dtype)
    ne = len(toks)
    pos = 0
    for TT in tok_tiles:
        take = toks[pos:pos + TT]
        if len(take):
            # [nt, 1024] -> [128, 8, nt]
            seg = xq[take].T.reshape(8, 128, len(take)).transpose(1, 0, 2)
            blk = out[:, 8 * pos:8 * (pos + TT)].reshape(128, 8, TT)
            blk[:, :, :len(take)] = seg
        pos += TT
    return out


def _make_in_maps(x, W1, b1, W2, b2, gates, order, tok_lists, C, tok_tiles):
    xh_full = _q8(x)
    xl_full = _q8(x - xh_full.astype(np.float32))
    in_maps = []
    for e in range(NUM_EXPERTS):
        toks = tok_lists[e]
        g_e = np.zeros(C, dtype=np.float32)
        g_e[:len(toks)] = gates[toks, e] / (S1 * S2)
        in_maps.append({
            "xh": _pack_x_tiles(xh_full, toks, tok_tiles, C),
            "xl": _pack_x_tiles(xl_full, toks, tok_tiles, C),
            "w1": _pack_w(W1[e], S1),
            "w2": _pack_w(W2[e], S2),
            "bb": np.ascontiguousarray(np.concatenate(
                [(S1 * b1[e]).reshape(8, 128).T,
                 (S1 * S2 * b2[e]).reshape(8, 128).T], axis=1)),
            "gb": np.ascontiguousarray(
                np.broadcast_to(g_e, (128, C))),
        })
    return in_maps


def kernel(x, W1, b1, W2, b2, Wg, bg):
    from concourse import bass_utils

    x = np.ascontiguousarray(np.asarray(x, dtype=np.float32))
    W1 = np.asarray(W1, dtype=np.float32)
    b1 = np.asarray(b1, dtype=np.float32)
    W2 = np.asarray(W2, dtype=np.float32)
    b2 = np.asarray(b2, dtype=np.float32)
    Wg = np.asarray(Wg, dtype=np.float32)
    bg = np.asarray(bg, dtype=np.float32)
    n = x.shape[0]

    gates, order = _route(x, Wg, bg)
    tok_lists = [np.where((order == e).any(axis=1))[0] for e in range(NUM_EXPERTS)]
    max_load = max(len(t) for t in tok_lists)
    C, tok_tiles = _plan_tiles(max_load)

    key = (C, tuple(tok_tiles))
    if key not in _prog_cache:
        _prog_cache[key] = _build_program((C, tok_tiles))
    nc = _prog_cache[key]

    in_maps = _make_in_maps(x, W1, b1, W2, b2, gates, order, tok_lists, C,
                            tok_tiles)
    res = bass_utils.run_bass_kernel_spmd(nc, in_maps, list(range(NUM_EXPERTS)))
    # yT result: tile-major [128, 8*C] bf16 -> [E, 128, 8, C] f32
    yT_all = np.empty((NUM_EXPERTS, 128, 8, C), dtype=np.float32)
    for e in range(NUM_EXPERTS):
        flat = res.results[e]["yT"].astype(np.float32)
        pos = 0
        for TT in tok_tiles:
            yT_all[e, :, :, pos:pos + TT] = (
                flat[:, 8 * pos:8 * (pos + TT)].reshape(128, 8, TT))
            pos += TT

    # scatter-add the two expert contributions per token (already gated)
    slot = np.zeros((NUM_EXPERTS, n), dtype=np.int64)
    for e in range(NUM_EXPERTS):
        slot[e, tok_lists[e]] = np.arange(len(tok_lists[e]))
    rows = np.arange(n)
    out = np.zeros((n, D), dtype=np.float32)
    for k in range(TOP_K):
        ek = order[:, k]
        picked = yT_all[ek, :, :, slot[ek, rows]]   # [n, 128, 8]
        out += picked.transpose(0, 2, 1).reshape(n, D)
    return out
